# revision 4
# baseline (speedup 1.0000x reference)
"""3x3 neighborhood (ADDA) attention on Trainium2, B=8, d=512 (8 heads x 64), 56x56.

Sharding: pure data parallel — batch b -> NeuronCore b (8 cores, SPMD, no
cross-core communication). Each core computes full local attention for one
batch.

v3 design (per core, 4 head-pair groups; partitions [0:116) = 2 heads x 58
padded rows, x padded to 60):

  HBM traffic 39MB -> ~24.7MB/core: v arrives as TWO x-parity variants only
  (not six dy x parity) — the window's row shift for the AV stage is folded
  into the TensorEngine accumulation, which uses three shifted-identity
  matrices A_dy instead of one identity:
      out[y] = sum_j A_dy(j) @ ( W_j[y-dy] * v[y] )
  where the partition-shifted weights W_j[y-dy] are two tiny [116,3,56]
  SBUF->SBUF DMAs per group.  k keeps three host-prepared dy variants (an
  on-chip partition-shift DMA measured ~41GB/s and sat on the critical
  path).  Loads are spread over three DMA queues (sync: q+k, gpsimd: v,
  scalar: weight shifts + output) so stores never block loads.

  QK stage: 9 DVE bf16 muls (2x mode) into a per-dy-batch [116,3,56,64]
  tile; the channel reduce is a pairwise tree with all 3 offsets of a batch
  fused per instruction (6 ops/batch instead of 18).  All elementwise work
  stays on DVE: GPSIMD compute shares an SBUF port with DVE and measured 5x
  slowdowns on concurrent DVE ops.  Softmax without max subtraction (logits
  are O(5)); 1/sqrt(64) folded into the ACT exp.  AV products are
  channel-outer so the per-pixel weight broadcasts along the outer free dim
  and mults stay in DVE 2x mode; `ta` is buffered 4-deep so the HAM-cold PE
  drains into the next group's QK phase instead of stalling DVE.
"""
import sys

sys.path.insert(0, "/opt/trn_rl_repo")

from contextlib import ExitStack

import ml_dtypes
import numpy as np

import concourse.bacc as bacc
import concourse.tile as tile
from concourse import mybir
from concourse.bass_utils import run_bass_kernel_spmd

B, D, H, W = 8, 512, 56, 56
NH, HD = 8, 64
SCALE = HD ** (-0.5)
N_CORES = 8
NG = 4          # head-pair groups
P58 = 58        # tile rows per head (1 + 56 + 1)
NP = 116        # compute partitions (2 heads x 58)
XT = 60         # padded x extent (even -> interiors stay 4B-aligned in bf16)
XI = 2          # interior x start
FLAT = HD * W   # 3584
BF16 = mybir.dt.bfloat16
F32 = mybir.dt.float32
BF = ml_dtypes.bfloat16

# j = 3*(dy+1) + (dx+1); dy-major so each batch of 3 shares one k variant.
OFFS = [(dy, dx) for dy in (-1, 0, 1) for dx in (-1, 0, 1)]

_NC_CACHE = {}


def _build_program():
    nc = bacc.Bacc("TRN2", target_bir_lowering=False, debug=False,
                   num_devices=N_CORES)
    q_d = nc.declare_dram_parameter("q", [NG, NP, XT, HD], BF16, isOutput=False)
    k_d = nc.declare_dram_parameter("k", [NG, NP, 3, XT, HD], BF16,
                                    isOutput=False)
    v_d = nc.declare_dram_parameter("v", [NG, NP, 2, HD, XT], BF16,
                                    isOutput=False)
    a_d = nc.declare_dram_parameter("amat", [NP, 3, NP], BF16, isOutput=False)
    o_d = nc.declare_dram_parameter("out", [NG, NP, HD, W], BF16, isOutput=True)

    with tile.TileContext(nc) as tc:
        with ExitStack() as ctx:
            one_pool = ctx.enter_context(tc.tile_pool(name="one", bufs=1))
            q_pool = ctx.enter_context(tc.tile_pool(name="q", bufs=2))
            k_pool = ctx.enter_context(tc.tile_pool(name="k", bufs=2))
            v_pool = ctx.enter_context(tc.tile_pool(name="v", bufs=2))
            tm_pool = ctx.enter_context(tc.tile_pool(name="tm", bufs=1))
            tr_pool = ctx.enter_context(tc.tile_pool(name="tr", bufs=1))
            sm_pool = ctx.enter_context(tc.tile_pool(name="sm", bufs=2))
            wsh_pool = ctx.enter_context(tc.tile_pool(name="wsh", bufs=1))
            ta_pool = ctx.enter_context(tc.tile_pool(name="ta", bufs=4))
            ob_pool = ctx.enter_context(tc.tile_pool(name="ob", bufs=2))
            ps_pool = ctx.enter_context(
                tc.tile_pool(name="ps", bufs=1, space="PSUM"))

            amat = one_pool.tile([NP, 3, NP], BF16)
            nc.sync.dma_start(out=amat[:], in_=a_d[:])
            zrow = one_pool.tile([1, 3, W], BF16)
            nc.vector.memset(zrow[:], 0.0)

            # persistent shifted-weight tiles; edge rows that the per-group
            # shift DMAs never write are zero-filled once (the PE propagates
            # 0*NaN, so every ta row must stay finite).
            wm1 = wsh_pool.tile([NP, 3, W], BF16, tag="wm1")
            wp1 = wsh_pool.tile([NP, 3, W], BF16, tag="wp1")
            nc.scalar.dma_start(out=wm1[NP - 1:NP], in_=zrow[:])
            nc.scalar.dma_start(out=wp1[0:1], in_=zrow[:])

            for g in range(NG):
                qt = q_pool.tile([NP, XT, HD], BF16, tag="qt")
                nc.sync.dma_start(out=qt[:], in_=q_d[g])
                ka = k_pool.tile([NP, 3, XT, HD], BF16, tag="ka")
                nc.sync.dma_start(out=ka[:], in_=k_d[g])
                va = v_pool.tile([NP, 2, HD, XT], BF16, tag="va")
                nc.gpsimd.dma_start(out=va[:], in_=v_d[g])

                L = sm_pool.tile([NP, 9, W], F32, tag="L")
                Pt = sm_pool.tile([NP, 9, W], BF16, tag="P")
                Wt = sm_pool.tile([NP, 9, W], BF16, tag="W")
                S = sm_pool.tile([NP, W], F32, tag="S")
                R = sm_pool.tile([NP, W], F32, tag="R")

                # --- QK: logits; batched pairwise channel-reduce tree ---
                for b in range(3):
                    tm = tm_pool.tile([NP, 3, W, HD], BF16, tag="tm")
                    for ji in range(3):
                        dy, dx = OFFS[3 * b + ji]
                        nc.vector.tensor_mul(
                            tm[:, ji, :, :],
                            qt[:, XI:XI + W, :],
                            ka[:, b, XI + dx:XI + dx + W, :],
                        )
                    t32 = tr_pool.tile([NP, 3, W, 32], BF16, tag="t32")
                    nc.vector.tensor_add(t32[:], tm[:, :, :, 0:32],
                                         tm[:, :, :, 32:64])
                    t16 = tr_pool.tile([NP, 3, W, 16], BF16, tag="t16")
                    nc.vector.tensor_add(t16[:], t32[:, :, :, 0:16],
                                         t32[:, :, :, 16:32])
                    t8 = tr_pool.tile([NP, 3, W, 8], BF16, tag="t8")
                    nc.vector.tensor_add(t8[:], t16[:, :, :, 0:8],
                                         t16[:, :, :, 8:16])
                    t4 = tr_pool.tile([NP, 3, W, 4], BF16, tag="t4")
                    nc.vector.tensor_add(t4[:], t8[:, :, :, 0:4],
                                         t8[:, :, :, 4:8])
                    t2 = tr_pool.tile([NP, 3, W, 2], F32, tag="t2")
                    nc.vector.tensor_add(t2[:], t4[:, :, :, 0:2],
                                         t4[:, :, :, 2:4])
                    nc.vector.tensor_add(L[:, 3 * b:3 * b + 3, :],
                                         t2[:, :, :, 0], t2[:, :, :, 1])

                # --- softmax (no max subtraction; SCALE folded into exp) ---
                nc.scalar.activation(
                    out=Pt[:, :, :], in_=L[:, :, :],
                    func=mybir.ActivationFunctionType.Exp, scale=float(SCALE),
                )
                nc.vector.tensor_reduce(
                    out=S[:, :],
                    in_=Pt[:, :, :].transpose([0, 2, 1]),
                    axis=mybir.AxisListType.X,
                    op=mybir.AluOpType.add,
                )
                nc.vector.reciprocal(out=R[:, :], in_=S[:, :])
                nc.vector.tensor_mul(
                    Wt[:, :, :],
                    Pt[:, :, :],
                    R[:, :].unsqueeze(1).to_broadcast((NP, 9, W)),
                )

                # shifted weights for the dy=+-1 AV batches:
                # wm1[p] = W[p+1] (dy=-1), wp1[p] = W[p-1] (dy=+1)
                nc.scalar.dma_start(out=wm1[0:NP - 1], in_=Wt[1:NP, 0:3, :])
                nc.scalar.dma_start(out=wp1[1:NP], in_=Wt[0:NP - 1, 6:9, :])

                # --- AV: dy=0 first (overlaps the wm1/wp1 DMAs); PE
                # accumulates with the A_dy shift matrices. ---
                av = ps_pool.tile([NP, FLAT], F32, tag="av")
                pos = 0
                for b in (1, 0, 2):
                    for ji in range(3):
                        j = 3 * b + ji
                        dy, dx = OFFS[j]
                        xp = dx & 1
                        xb = XI + xp + dx
                        if dy == 0:
                            w_ap = Wt[:, j:j + 1, :]
                        elif dy == -1:
                            w_ap = wm1[:, ji:ji + 1, :]
                        else:
                            w_ap = wp1[:, ji:ji + 1, :]
                        ta = ta_pool.tile([NP, HD, W], BF16, tag="ta")
                        nc.vector.tensor_mul(
                            ta[:, :, :],
                            w_ap.to_broadcast((NP, HD, W)),
                            va[:, xp, :, xb:xb + W],
                        )
                        taf = ta[:, :, :].rearrange("p c x -> p (c x)")
                        for ch in range(FLAT // 512):
                            nc.tensor.matmul(
                                av[:, ch * 512:(ch + 1) * 512],
                                amat[:, b, :],
                                taf[:, ch * 512:(ch + 1) * 512],
                                start=(pos == 0),
                                stop=(pos == 8),
                            )
                        pos += 1

                ob = ob_pool.tile([NP, HD, W], BF16, tag="ob")
                nc.scalar.copy(ob[:, :, :], av[:, :].rearrange(
                    "p (c x) -> p c x", c=HD))
                nc.scalar.dma_start(out=o_d[g], in_=ob[:])

    nc.compile()
    return nc


def _get_nc():
    if "nc" not in _NC_CACHE:
        _NC_CACHE["nc"] = _build_program()
    return _NC_CACHE["nc"]


def _prep_inputs(q, k, v):
    """Build per-core images (leading dim = core/batch).

    q: [B, NG, 116, 60, 64]; k: [B, NG, 116, 3, 60, 64] (dy in {-1,0,1});
    v: [B, NG, 116, 2, 64, 60] (x-parity variants).
    Tile row p = hh*58 + pr holds image row y = pr - 1 (+dy for k variants);
    out-of-range rows and x pads are zero.  amat[p, d, y] = 1 iff ta-row p
    feeds out-row y for dy = d-1 (y = p - dy), edge rows routed to their own
    (pad) row.
    """
    qyxc = q.reshape(B, NH, HD, H, W).transpose(0, 1, 3, 4, 2).astype(BF)
    kyxc = k.reshape(B, NH, HD, H, W).transpose(0, 1, 3, 4, 2).astype(BF)
    vycx = v.reshape(B, NH, HD, H, W).transpose(0, 1, 3, 2, 4).astype(BF)

    qi = np.zeros((B, NG, NP, XT, HD), dtype=BF)
    ki = np.zeros((B, NG, NP, 3, XT, HD), dtype=BF)
    vi = np.zeros((B, NG, NP, 2, HD, XT), dtype=BF)
    for g in range(NG):
        for hh in range(2):
            hd = 2 * g + hh
            p0 = hh * P58
            qi[:, g, p0 + 1:p0 + 1 + H, XI:XI + W, :] = qyxc[:, hd]
            for di, dy in enumerate((-1, 0, 1)):
                a, b = max(0, 1 - dy), min(P58, P58 - 1 - dy)
                ki[:, g, p0 + a:p0 + b, di, XI:XI + W, :] = \
                    kyxc[:, hd, a - 1 + dy:b - 1 + dy]
            for xp in (0, 1):
                vi[:, g, p0 + 1:p0 + 1 + H, xp, :, XI + xp:XI + xp + W] = \
                    vycx[:, hd]
    amat = np.zeros((NP, 3, NP), dtype=BF)
    for d, dy in enumerate((-1, 0, 1)):
        for p in range(NP):
            y = p - dy
            amat[p, d, y if 0 <= y < NP else p] = 1
    return [{"q": qi[b], "k": ki[b], "v": vi[b], "amat": amat}
            for b in range(N_CORES)]


def _run(q, k, v, trace=False, tmpdir=None):
    q = np.asarray(q, dtype=np.float32)
    k = np.asarray(k, dtype=np.float32)
    v = np.asarray(v, dtype=np.float32)
    in_maps = _prep_inputs(q, k, v)
    nc = _get_nc()
    res = run_bass_kernel_spmd(nc, in_maps, core_ids=list(range(N_CORES)),
                               trace=trace, tmpdir=tmpdir)
    # out image [NG, 116, 64, 56] -> [y, x, c]
    out = np.empty((B, H, W, D), dtype=np.float32)
    for b in range(N_CORES):
        oi = np.asarray(res.results[b]["out"]).astype(np.float32)
        for g in range(NG):
            for hh in range(2):
                hd = 2 * g + hh
                blk = oi[g, hh * P58 + 1:hh * P58 + 1 + H]     # [y, c, x]
                out[b, :, :, hd * HD:(hd + 1) * HD] = blk.transpose(0, 2, 1)
    return out, res


def kernel(q, k, v):
    out, _ = _run(q, k, v, trace=False)
    return out


def run_traced(q, k, v, tmpdir=None):
    out, res = _run(q, k, v, trace=True, tmpdir=tmpdir)
    return out, res


# revision 7
# speedup vs baseline: 1.0924x; 1.0924x over previous
"""3x3 neighborhood (ADDA) attention on Trainium2, B=8, d=512 (8 heads x 64), 56x56.

Sharding: pure data parallel — batch b -> NeuronCore b (8 cores, SPMD, no
cross-core communication). Each core computes full local attention for one
batch.

v3 design (per core, 4 head-pair groups; partitions [0:116) = 2 heads x 58
padded rows, x padded to 60):

  HBM traffic 39MB -> ~24.7MB/core: v arrives as TWO x-parity variants only
  (not six dy x parity) — the window's row shift for the AV stage is folded
  into the TensorEngine accumulation, which uses three shifted-identity
  matrices A_dy instead of one identity:
      out[y] = sum_j A_dy(j) @ ( W_j[y-dy] * v[y] )
  where the partition-shifted weights W_j[y-dy] are two tiny [116,3,56]
  SBUF->SBUF DMAs per group.  k keeps three host-prepared dy variants (an
  on-chip partition-shift DMA measured ~41GB/s and sat on the critical
  path).  Loads are spread over three DMA queues (sync: q+k, gpsimd: v,
  scalar: weight shifts + output) so stores never block loads.

  QK stage: 9 DVE bf16 muls (2x mode) into a per-dy-batch [116,3,56,64]
  tile; the channel reduce is a pairwise tree with all 3 offsets of a batch
  fused per instruction (6 ops/batch instead of 18).  All elementwise work
  stays on DVE: GPSIMD compute shares an SBUF port with DVE and measured 5x
  slowdowns on concurrent DVE ops.  Softmax without max subtraction (logits
  are O(5)); 1/sqrt(64) folded into the ACT exp.  AV products are
  channel-outer so the per-pixel weight broadcasts along the outer free dim
  and mults stay in DVE 2x mode; `ta` is buffered 4-deep so the HAM-cold PE
  drains into the next group's QK phase instead of stalling DVE.
"""
import sys

sys.path.insert(0, "/opt/trn_rl_repo")

from contextlib import ExitStack

import ml_dtypes
import numpy as np

import concourse.bacc as bacc
import concourse.tile as tile
from concourse import mybir
from concourse.bass_utils import run_bass_kernel_spmd

B, D, H, W = 8, 512, 56, 56
NH, HD = 8, 64
SCALE = HD ** (-0.5)
N_CORES = 8
NG = 4          # head-pair groups
P58 = 58        # tile rows per head (1 + 56 + 1)
NP = 116        # compute partitions (2 heads x 58)
XT = 60         # padded x extent (even -> interiors stay 4B-aligned in bf16)
XI = 2          # interior x start
FLAT = HD * W   # 3584
BF16 = mybir.dt.bfloat16
F32 = mybir.dt.float32
BF = ml_dtypes.bfloat16

# j = 3*(dy+1) + (dx+1); dy-major so each batch of 3 shares one k variant.
OFFS = [(dy, dx) for dy in (-1, 0, 1) for dx in (-1, 0, 1)]

_NC_CACHE = {}


def _build_program():
    nc = bacc.Bacc("TRN2", target_bir_lowering=False, debug=False,
                   num_devices=N_CORES)
    q_d = nc.declare_dram_parameter("q", [NG, NP, XT, HD], BF16, isOutput=False)
    k_d = nc.declare_dram_parameter("k", [NG, NP, 3, XT, HD], BF16,
                                    isOutput=False)
    v_d = nc.declare_dram_parameter("v", [NG, NP, 2, HD, XT], BF16,
                                    isOutput=False)
    a_d = nc.declare_dram_parameter("amat", [NP, 3, NP], BF16, isOutput=False)
    o_d = nc.declare_dram_parameter("out", [NG, NP, HD, W], BF16, isOutput=True)

    with tile.TileContext(nc) as tc:
        with ExitStack() as ctx:
            one_pool = ctx.enter_context(tc.tile_pool(name="one", bufs=1))
            q_pool = ctx.enter_context(tc.tile_pool(name="q", bufs=2))
            k_pool = ctx.enter_context(tc.tile_pool(name="k", bufs=2))
            v_pool = ctx.enter_context(tc.tile_pool(name="v", bufs=2))
            tm_pool = ctx.enter_context(tc.tile_pool(name="tm", bufs=1))
            tr_pool = ctx.enter_context(tc.tile_pool(name="tr", bufs=1))
            sm_pool = ctx.enter_context(tc.tile_pool(name="sm", bufs=2))
            wsh_pool = ctx.enter_context(tc.tile_pool(name="wsh", bufs=1))
            ta_pool = ctx.enter_context(tc.tile_pool(name="ta", bufs=4))
            ob_pool = ctx.enter_context(tc.tile_pool(name="ob", bufs=2))
            ps_pool = ctx.enter_context(
                tc.tile_pool(name="ps", bufs=1, space="PSUM"))

            amat = one_pool.tile([NP, 3, NP], BF16)
            nc.sync.dma_start(out=amat[:], in_=a_d[:])
            zrow = one_pool.tile([1, 3, W], BF16)
            nc.vector.memset(zrow[:], 0.0)

            # persistent shifted-weight tiles; edge rows that the per-group
            # shift DMAs never write are zero-filled once (the PE propagates
            # 0*NaN, so every ta row must stay finite).
            wm1 = wsh_pool.tile([NP, 3, W], BF16, tag="wm1")
            wp1 = wsh_pool.tile([NP, 3, W], BF16, tag="wp1")
            nc.scalar.dma_start(out=wm1[NP - 1:NP], in_=zrow[:])
            nc.scalar.dma_start(out=wp1[0:1], in_=zrow[:])

            def emit_loads(g):
                """Issue group g's six 0.87MB loads spread over the three
                DMA rings (sync/scalar/gpsimd).  Group 0 puts the dy=0 QK
                operands first on separate rings so the first muls start
                ~12us in; later groups balance bytes per ring."""
                qt = q_pool.tile([NP, XT, HD], BF16, tag="qt")
                ka = k_pool.tile([NP, 3, XT, HD], BF16, tag="ka")
                va = v_pool.tile([NP, 2, HD, XT], BF16, tag="va")
                if g == 0:
                    nc.sync.dma_start(out=qt[:], in_=q_d[g])
                    nc.scalar.dma_start(out=ka[:, 1], in_=k_d[g, :, 1])
                    nc.sync.dma_start(out=ka[:, 0], in_=k_d[g, :, 0])
                    nc.gpsimd.dma_start(out=ka[:, 2], in_=k_d[g, :, 2])
                    nc.scalar.dma_start(out=va[:, 0], in_=v_d[g, :, 0])
                    nc.gpsimd.dma_start(out=va[:, 1], in_=v_d[g, :, 1])
                else:
                    nc.sync.dma_start(out=qt[:], in_=q_d[g])
                    nc.sync.dma_start(out=ka[:, 1], in_=k_d[g, :, 1])
                    nc.scalar.dma_start(out=ka[:, 0], in_=k_d[g, :, 0])
                    nc.gpsimd.dma_start(out=ka[:, 2], in_=k_d[g, :, 2])
                    nc.scalar.dma_start(out=va[:, 0], in_=v_d[g, :, 0])
                    nc.gpsimd.dma_start(out=va[:, 1], in_=v_d[g, :, 1])
                return qt, ka, va

            tiles = [emit_loads(0), emit_loads(1)]

            for g in range(NG):
                qt, ka, va = tiles[g]

                L = sm_pool.tile([NP, 9, W], F32, tag="L")
                Pt = sm_pool.tile([NP, 9, W], BF16, tag="P")
                Wt = sm_pool.tile([NP, 9, W], BF16, tag="W")
                S = sm_pool.tile([NP, W], F32, tag="S")
                R = sm_pool.tile([NP, W], F32, tag="R")

                # --- QK: logits; batched pairwise channel-reduce tree.
                # dy=0 batch first: its k variant lands earliest. ---
                for b in (1, 0, 2):
                    tm = tm_pool.tile([NP, 3, W, HD], BF16, tag="tm")
                    for ji in range(3):
                        dy, dx = OFFS[3 * b + ji]
                        nc.vector.tensor_mul(
                            tm[:, ji, :, :],
                            qt[:, XI:XI + W, :],
                            ka[:, b, XI + dx:XI + dx + W, :],
                        )
                    t32 = tr_pool.tile([NP, 3, W, 32], BF16, tag="t32")
                    nc.vector.tensor_add(t32[:], tm[:, :, :, 0:32],
                                         tm[:, :, :, 32:64])
                    t16 = tr_pool.tile([NP, 3, W, 16], BF16, tag="t16")
                    nc.vector.tensor_add(t16[:], t32[:, :, :, 0:16],
                                         t32[:, :, :, 16:32])
                    t8 = tr_pool.tile([NP, 3, W, 8], BF16, tag="t8")
                    nc.vector.tensor_add(t8[:], t16[:, :, :, 0:8],
                                         t16[:, :, :, 8:16])
                    t4 = tr_pool.tile([NP, 3, W, 4], BF16, tag="t4")
                    nc.vector.tensor_add(t4[:], t8[:, :, :, 0:4],
                                         t8[:, :, :, 4:8])
                    t2 = tr_pool.tile([NP, 3, W, 2], F32, tag="t2")
                    nc.vector.tensor_add(t2[:], t4[:, :, :, 0:2],
                                         t4[:, :, :, 2:4])
                    nc.vector.tensor_add(L[:, 3 * b:3 * b + 3, :],
                                         t2[:, :, :, 0], t2[:, :, :, 1])

                # --- softmax (no max subtraction; SCALE folded into exp) ---
                nc.scalar.activation(
                    out=Pt[:, :, :], in_=L[:, :, :],
                    func=mybir.ActivationFunctionType.Exp, scale=float(SCALE),
                )
                nc.vector.tensor_reduce(
                    out=S[:, :],
                    in_=Pt[:, :, :].transpose([0, 2, 1]),
                    axis=mybir.AxisListType.X,
                    op=mybir.AluOpType.add,
                )
                nc.vector.reciprocal(out=R[:, :], in_=S[:, :])
                nc.vector.tensor_mul(
                    Wt[:, :, :],
                    Pt[:, :, :],
                    R[:, :].unsqueeze(1).to_broadcast((NP, 9, W)),
                )

                # shifted weights for the dy=+-1 AV batches:
                # wm1[p] = W[p+1] (dy=-1), wp1[p] = W[p-1] (dy=+1)
                nc.scalar.dma_start(out=wm1[0:NP - 1], in_=Wt[1:NP, 0:3, :])
                nc.scalar.dma_start(out=wp1[1:NP], in_=Wt[0:NP - 1, 6:9, :])

                # --- AV: dy=0 first (overlaps the wm1/wp1 DMAs); PE
                # accumulates with the A_dy shift matrices. ---
                av = ps_pool.tile([NP, FLAT], F32, tag="av")
                pos = 0
                for b in (1, 0, 2):
                    for ji in range(3):
                        j = 3 * b + ji
                        dy, dx = OFFS[j]
                        xp = dx & 1
                        xb = XI + xp + dx
                        if dy == 0:
                            w_ap = Wt[:, j:j + 1, :]
                        elif dy == -1:
                            w_ap = wm1[:, ji:ji + 1, :]
                        else:
                            w_ap = wp1[:, ji:ji + 1, :]
                        ta = ta_pool.tile([NP, HD, W], BF16, tag="ta")
                        nc.vector.tensor_mul(
                            ta[:, :, :],
                            w_ap.to_broadcast((NP, HD, W)),
                            va[:, xp, :, xb:xb + W],
                        )
                        taf = ta[:, :, :].rearrange("p c x -> p (c x)")
                        for ch in range(FLAT // 512):
                            nc.tensor.matmul(
                                av[:, ch * 512:(ch + 1) * 512],
                                amat[:, b, :],
                                taf[:, ch * 512:(ch + 1) * 512],
                                start=(pos == 0),
                                stop=(pos == 8),
                            )
                        pos += 1

                ob = ob_pool.tile([NP, HD, W], BF16, tag="ob")
                nc.scalar.copy(ob[:, :, :], av[:, :].rearrange(
                    "p (c x) -> p c x", c=HD))
                nc.gpsimd.dma_start(out=o_d[g], in_=ob[:])
                if g + 2 < NG:
                    tiles.append(emit_loads(g + 2))

    nc.compile()
    return nc


def _get_nc():
    if "nc" not in _NC_CACHE:
        _NC_CACHE["nc"] = _build_program()
    return _NC_CACHE["nc"]


def _prep_inputs(q, k, v):
    """Build per-core images (leading dim = core/batch).

    q: [B, NG, 116, 60, 64]; k: [B, NG, 116, 3, 60, 64] (dy in {-1,0,1});
    v: [B, NG, 116, 2, 64, 60] (x-parity variants).
    Tile row p = hh*58 + pr holds image row y = pr - 1 (+dy for k variants);
    out-of-range rows and x pads are zero.  amat[p, d, y] = 1 iff ta-row p
    feeds out-row y for dy = d-1 (y = p - dy), edge rows routed to their own
    (pad) row.
    """
    qyxc = q.reshape(B, NH, HD, H, W).transpose(0, 1, 3, 4, 2).astype(BF)
    kyxc = k.reshape(B, NH, HD, H, W).transpose(0, 1, 3, 4, 2).astype(BF)
    vycx = v.reshape(B, NH, HD, H, W).transpose(0, 1, 3, 2, 4).astype(BF)

    qi = np.zeros((B, NG, NP, XT, HD), dtype=BF)
    ki = np.zeros((B, NG, NP, 3, XT, HD), dtype=BF)
    vi = np.zeros((B, NG, NP, 2, HD, XT), dtype=BF)
    for g in range(NG):
        for hh in range(2):
            hd = 2 * g + hh
            p0 = hh * P58
            qi[:, g, p0 + 1:p0 + 1 + H, XI:XI + W, :] = qyxc[:, hd]
            for di, dy in enumerate((-1, 0, 1)):
                a, b = max(0, 1 - dy), min(P58, P58 - 1 - dy)
                ki[:, g, p0 + a:p0 + b, di, XI:XI + W, :] = \
                    kyxc[:, hd, a - 1 + dy:b - 1 + dy]
            for xp in (0, 1):
                vi[:, g, p0 + 1:p0 + 1 + H, xp, :, XI + xp:XI + xp + W] = \
                    vycx[:, hd]
    amat = np.zeros((NP, 3, NP), dtype=BF)
    for d, dy in enumerate((-1, 0, 1)):
        for p in range(NP):
            y = p - dy
            amat[p, d, y if 0 <= y < NP else p] = 1
    return [{"q": qi[b], "k": ki[b], "v": vi[b], "amat": amat}
            for b in range(N_CORES)]


def _run(q, k, v, trace=False, tmpdir=None):
    q = np.asarray(q, dtype=np.float32)
    k = np.asarray(k, dtype=np.float32)
    v = np.asarray(v, dtype=np.float32)
    in_maps = _prep_inputs(q, k, v)
    nc = _get_nc()
    res = run_bass_kernel_spmd(nc, in_maps, core_ids=list(range(N_CORES)),
                               trace=trace, tmpdir=tmpdir)
    # out image [NG, 116, 64, 56] -> [y, x, c]
    out = np.empty((B, H, W, D), dtype=np.float32)
    for b in range(N_CORES):
        oi = np.asarray(res.results[b]["out"]).astype(np.float32)
        for g in range(NG):
            for hh in range(2):
                hd = 2 * g + hh
                blk = oi[g, hh * P58 + 1:hh * P58 + 1 + H]     # [y, c, x]
                out[b, :, :, hd * HD:(hd + 1) * HD] = blk.transpose(0, 2, 1)
    return out, res


def kernel(q, k, v):
    out, _ = _run(q, k, v, trace=False)
    return out


def run_traced(q, k, v, tmpdir=None):
    out, res = _run(q, k, v, trace=True, tmpdir=tmpdir)
    return out, res


# revision 9
# speedup vs baseline: 1.1209x; 1.0261x over previous
"""3x3 neighborhood (ADDA) attention on Trainium2, B=8, d=512 (8 heads x 64), 56x56.

Sharding: pure data parallel — batch b -> NeuronCore b (8 cores, SPMD, no
cross-core communication). Each core computes full local attention for one
batch.

v3 design (per core, 4 head-pair groups; partitions [0:116) = 2 heads x 58
padded rows, x padded to 60):

  HBM traffic 39MB -> ~24.7MB/core: v arrives as TWO x-parity variants only
  (not six dy x parity) — the window's row shift for the AV stage is folded
  into the TensorEngine accumulation, which uses three shifted-identity
  matrices A_dy instead of one identity:
      out[y] = sum_j A_dy(j) @ ( W_j[y-dy] * v[y] )
  where the partition-shifted weights W_j[y-dy] are two tiny [116,3,56]
  SBUF->SBUF DMAs per group.  k keeps three host-prepared dy variants (an
  on-chip partition-shift DMA measured ~41GB/s and sat on the critical
  path).  Loads are spread over three DMA queues (sync: q+k, gpsimd: v,
  scalar: weight shifts + output) so stores never block loads.

  QK stage: 9 DVE bf16 muls (2x mode) into a per-dy-batch [116,3,56,64]
  tile; the channel reduce is a pairwise tree with all 3 offsets of a batch
  fused per instruction (6 ops/batch instead of 18).  All elementwise work
  stays on DVE: GPSIMD compute shares an SBUF port with DVE and measured 5x
  slowdowns on concurrent DVE ops.  Softmax without max subtraction (logits
  are O(5)); 1/sqrt(64) folded into the ACT exp.  AV products are
  channel-outer so the per-pixel weight broadcasts along the outer free dim
  and mults stay in DVE 2x mode; `ta` is buffered 4-deep so the HAM-cold PE
  drains into the next group's QK phase instead of stalling DVE.
"""
import sys

sys.path.insert(0, "/opt/trn_rl_repo")

from contextlib import ExitStack

import ml_dtypes
import numpy as np

import concourse.bacc as bacc
import concourse.tile as tile
from concourse import mybir
from concourse.bass_utils import run_bass_kernel_spmd

B, D, H, W = 8, 512, 56, 56
NH, HD = 8, 64
SCALE = HD ** (-0.5)
N_CORES = 8
NG = 4          # head-pair groups
P58 = 58        # tile rows per head (1 + 56 + 1)
NP = 116        # compute partitions (2 heads x 58)
XT = 60         # padded x extent (even -> interiors stay 4B-aligned in bf16)
XI = 2          # interior x start
FLAT = HD * W   # 3584
BF16 = mybir.dt.bfloat16
F32 = mybir.dt.float32
BF = ml_dtypes.bfloat16

# j = 3*(dy+1) + (dx+1); dy-major so each batch of 3 shares one k variant.
OFFS = [(dy, dx) for dy in (-1, 0, 1) for dx in (-1, 0, 1)]

_NC_CACHE = {}


def _build_program():
    nc = bacc.Bacc("TRN2", target_bir_lowering=False, debug=False,
                   num_devices=N_CORES)
    q_d = nc.declare_dram_parameter("q", [NG, NP, XT, HD], BF16, isOutput=False)
    k_d = nc.declare_dram_parameter("k", [NG, NP, 3, XT, HD], BF16,
                                    isOutput=False)
    v_d = nc.declare_dram_parameter("v", [NG, NP, 2, HD, XT], BF16,
                                    isOutput=False)
    a_d = nc.declare_dram_parameter("amat", [NP, 3, NP], BF16, isOutput=False)
    o_d = nc.declare_dram_parameter("out", [NG, NP, HD, W], BF16, isOutput=True)

    with tile.TileContext(nc) as tc:
        with ExitStack() as ctx:
            one_pool = ctx.enter_context(tc.tile_pool(name="one", bufs=1))
            q_pool = ctx.enter_context(tc.tile_pool(name="q", bufs=2))
            k_pool = ctx.enter_context(tc.tile_pool(name="k", bufs=2))
            v_pool = ctx.enter_context(tc.tile_pool(name="v", bufs=2))
            tm_pool = ctx.enter_context(tc.tile_pool(name="tm", bufs=1))
            tr_pool = ctx.enter_context(tc.tile_pool(name="tr", bufs=1))
            sm_pool = ctx.enter_context(tc.tile_pool(name="sm", bufs=2))
            wsh_pool = ctx.enter_context(tc.tile_pool(name="wsh", bufs=1))
            ta_pool = ctx.enter_context(tc.tile_pool(name="ta", bufs=4))
            ob_pool = ctx.enter_context(tc.tile_pool(name="ob", bufs=2))
            ps_pool = ctx.enter_context(
                tc.tile_pool(name="ps", bufs=1, space="PSUM"))

            amat = one_pool.tile([NP, 3, NP], BF16)
            nc.sync.dma_start(out=amat[:], in_=a_d[:])
            zrow = one_pool.tile([1, 3, W], BF16)
            nc.vector.memset(zrow[:], 0.0)

            # persistent shifted-weight tiles; edge rows that the per-group
            # shift DMAs never write are zero-filled once (the PE propagates
            # 0*NaN, so every ta row must stay finite).
            wm1 = wsh_pool.tile([NP, 3, W], BF16, tag="wm1")
            wp1 = wsh_pool.tile([NP, 3, W], BF16, tag="wp1")
            nc.scalar.dma_start(out=wm1[NP - 1:NP], in_=zrow[:])
            nc.scalar.dma_start(out=wp1[0:1], in_=zrow[:])

            gate = one_pool.tile([1, 2, 2], BF16)

            def emit_loads(g):
                """Issue group g's six 0.87MB loads spread over the three
                DMA rings (sync/scalar/gpsimd).  For group 0, only the two
                tensors the first muls need (q + dy=0 k variant) flow
                immediately; the other rings are gated behind the q load by
                a tiny dependent DMA so the critical path gets the whole
                (8-core-contended) HBM bandwidth."""
                qt = q_pool.tile([NP, XT, HD], BF16, tag="qt")
                ka = k_pool.tile([NP, 3, XT, HD], BF16, tag="ka")
                va = v_pool.tile([NP, 2, HD, XT], BF16, tag="va")
                nc.sync.dma_start(out=qt[:], in_=q_d[g])
                nc.scalar.dma_start(out=ka[:, 1], in_=k_d[g, :, 1])
                if g == 0:
                    nc.scalar.dma_start(out=gate[0:1, 0], in_=qt[0:1, 0, 0:2])
                    nc.gpsimd.dma_start(out=gate[0:1, 1], in_=qt[0:1, 0, 0:2])
                nc.sync.dma_start(out=ka[:, 0], in_=k_d[g, :, 0])
                nc.gpsimd.dma_start(out=ka[:, 2], in_=k_d[g, :, 2])
                nc.scalar.dma_start(out=va[:, 0], in_=v_d[g, :, 0])
                nc.gpsimd.dma_start(out=va[:, 1], in_=v_d[g, :, 1])
                return qt, ka, va

            tiles = [emit_loads(0), emit_loads(1)]

            for g in range(NG):
                qt, ka, va = tiles[g]

                L = sm_pool.tile([NP, 9, W], F32, tag="L")
                Pt = sm_pool.tile([NP, 9, W], BF16, tag="P")
                Wt = sm_pool.tile([NP, 9, W], BF16, tag="W")
                S = sm_pool.tile([NP, W], F32, tag="S")
                R = sm_pool.tile([NP, W], F32, tag="R")

                # --- QK: logits; batched pairwise channel-reduce tree.
                # dy=0 batch first: its k variant lands earliest. ---
                for b in (1, 0, 2):
                    tm = tm_pool.tile([NP, 3, W, HD], BF16, tag="tm")
                    for ji in range(3):
                        dy, dx = OFFS[3 * b + ji]
                        nc.vector.tensor_mul(
                            tm[:, ji, :, :],
                            qt[:, XI:XI + W, :],
                            ka[:, b, XI + dx:XI + dx + W, :],
                        )
                    t32 = tr_pool.tile([NP, 3, W, 32], BF16, tag="t32")
                    nc.vector.tensor_add(t32[:], tm[:, :, :, 0:32],
                                         tm[:, :, :, 32:64])
                    t16 = tr_pool.tile([NP, 3, W, 16], BF16, tag="t16")
                    nc.vector.tensor_add(t16[:], t32[:, :, :, 0:16],
                                         t32[:, :, :, 16:32])
                    t8 = tr_pool.tile([NP, 3, W, 8], BF16, tag="t8")
                    nc.vector.tensor_add(t8[:], t16[:, :, :, 0:8],
                                         t16[:, :, :, 8:16])
                    t4 = tr_pool.tile([NP, 3, W, 4], BF16, tag="t4")
                    nc.vector.tensor_add(t4[:], t8[:, :, :, 0:4],
                                         t8[:, :, :, 4:8])
                    t2 = tr_pool.tile([NP, 3, W, 2], F32, tag="t2")
                    nc.vector.tensor_add(t2[:], t4[:, :, :, 0:2],
                                         t4[:, :, :, 2:4])
                    nc.vector.tensor_add(L[:, 3 * b:3 * b + 3, :],
                                         t2[:, :, :, 0], t2[:, :, :, 1])

                # --- softmax (no max subtraction; SCALE folded into exp) ---
                nc.scalar.activation(
                    out=Pt[:, :, :], in_=L[:, :, :],
                    func=mybir.ActivationFunctionType.Exp, scale=float(SCALE),
                )
                nc.vector.tensor_reduce(
                    out=S[:, :],
                    in_=Pt[:, :, :].transpose([0, 2, 1]),
                    axis=mybir.AxisListType.X,
                    op=mybir.AluOpType.add,
                )
                nc.vector.reciprocal(out=R[:, :], in_=S[:, :])
                nc.vector.tensor_mul(
                    Wt[:, :, :],
                    Pt[:, :, :],
                    R[:, :].unsqueeze(1).to_broadcast((NP, 9, W)),
                )

                # shifted weights for the dy=+-1 AV batches:
                # wm1[p] = W[p+1] (dy=-1), wp1[p] = W[p-1] (dy=+1)
                nc.scalar.dma_start(out=wm1[0:NP - 1], in_=Wt[1:NP, 0:3, :])
                nc.scalar.dma_start(out=wp1[1:NP], in_=Wt[0:NP - 1, 6:9, :])

                # --- AV: dy=0 first (overlaps the wm1/wp1 DMAs); PE
                # accumulates with the A_dy shift matrices. ---
                av = ps_pool.tile([NP, FLAT], F32, tag="av")
                pos = 0
                for b in (1, 0, 2):
                    for ji in range(3):
                        j = 3 * b + ji
                        dy, dx = OFFS[j]
                        xp = dx & 1
                        xb = XI + xp + dx
                        if dy == 0:
                            w_ap = Wt[:, j:j + 1, :]
                        elif dy == -1:
                            w_ap = wm1[:, ji:ji + 1, :]
                        else:
                            w_ap = wp1[:, ji:ji + 1, :]
                        ta = ta_pool.tile([NP, HD, W], BF16, tag="ta")
                        nc.vector.tensor_mul(
                            ta[:, :, :],
                            w_ap.to_broadcast((NP, HD, W)),
                            va[:, xp, :, xb:xb + W],
                        )
                        taf = ta[:, :, :].rearrange("p c x -> p (c x)")
                        for ch in range(FLAT // 512):
                            nc.tensor.matmul(
                                av[:, ch * 512:(ch + 1) * 512],
                                amat[:, b, :],
                                taf[:, ch * 512:(ch + 1) * 512],
                                start=(pos == 0),
                                stop=(pos == 8),
                            )
                        pos += 1

                ob = ob_pool.tile([NP, HD, W], BF16, tag="ob")
                nc.scalar.copy(ob[:, :, :], av[:, :].rearrange(
                    "p (c x) -> p c x", c=HD))
                nc.sync.dma_start(out=o_d[g], in_=ob[:])
                if g + 2 < NG:
                    tiles.append(emit_loads(g + 2))

    nc.compile()
    return nc


def _get_nc():
    if "nc" not in _NC_CACHE:
        _NC_CACHE["nc"] = _build_program()
    return _NC_CACHE["nc"]


def _prep_inputs(q, k, v):
    """Build per-core images (leading dim = core/batch).

    q: [B, NG, 116, 60, 64]; k: [B, NG, 116, 3, 60, 64] (dy in {-1,0,1});
    v: [B, NG, 116, 2, 64, 60] (x-parity variants).
    Tile row p = hh*58 + pr holds image row y = pr - 1 (+dy for k variants);
    out-of-range rows and x pads are zero.  amat[p, d, y] = 1 iff ta-row p
    feeds out-row y for dy = d-1 (y = p - dy), edge rows routed to their own
    (pad) row.
    """
    qyxc = q.reshape(B, NH, HD, H, W).transpose(0, 1, 3, 4, 2).astype(BF)
    kyxc = k.reshape(B, NH, HD, H, W).transpose(0, 1, 3, 4, 2).astype(BF)
    vycx = v.reshape(B, NH, HD, H, W).transpose(0, 1, 3, 2, 4).astype(BF)

    qi = np.zeros((B, NG, NP, XT, HD), dtype=BF)
    ki = np.zeros((B, NG, NP, 3, XT, HD), dtype=BF)
    vi = np.zeros((B, NG, NP, 2, HD, XT), dtype=BF)
    for g in range(NG):
        for hh in range(2):
            hd = 2 * g + hh
            p0 = hh * P58
            qi[:, g, p0 + 1:p0 + 1 + H, XI:XI + W, :] = qyxc[:, hd]
            for di, dy in enumerate((-1, 0, 1)):
                a, b = max(0, 1 - dy), min(P58, P58 - 1 - dy)
                ki[:, g, p0 + a:p0 + b, di, XI:XI + W, :] = \
                    kyxc[:, hd, a - 1 + dy:b - 1 + dy]
            for xp in (0, 1):
                vi[:, g, p0 + 1:p0 + 1 + H, xp, :, XI + xp:XI + xp + W] = \
                    vycx[:, hd]
    amat = np.zeros((NP, 3, NP), dtype=BF)
    for d, dy in enumerate((-1, 0, 1)):
        for p in range(NP):
            y = p - dy
            amat[p, d, y if 0 <= y < NP else p] = 1
    return [{"q": qi[b], "k": ki[b], "v": vi[b], "amat": amat}
            for b in range(N_CORES)]


def _run(q, k, v, trace=False, tmpdir=None):
    q = np.asarray(q, dtype=np.float32)
    k = np.asarray(k, dtype=np.float32)
    v = np.asarray(v, dtype=np.float32)
    in_maps = _prep_inputs(q, k, v)
    nc = _get_nc()
    res = run_bass_kernel_spmd(nc, in_maps, core_ids=list(range(N_CORES)),
                               trace=trace, tmpdir=tmpdir)
    # out image [NG, 116, 64, 56] -> [y, x, c]
    out = np.empty((B, H, W, D), dtype=np.float32)
    for b in range(N_CORES):
        oi = np.asarray(res.results[b]["out"]).astype(np.float32)
        for g in range(NG):
            for hh in range(2):
                hd = 2 * g + hh
                blk = oi[g, hh * P58 + 1:hh * P58 + 1 + H]     # [y, c, x]
                out[b, :, :, hd * HD:(hd + 1) * HD] = blk.transpose(0, 2, 1)
    return out, res


def kernel(q, k, v):
    out, _ = _run(q, k, v, trace=False)
    return out


def run_traced(q, k, v, tmpdir=None):
    out, res = _run(q, k, v, trace=True, tmpdir=tmpdir)
    return out, res
